# revision 32
# baseline (speedup 1.0000x reference)
"""CenterLoss kernel for Trainium2 (Bass/Tile), data-parallel over 8 NeuronCores.

reference:
    d_i = ||x_i - c_{l_i}||^2 ;  loss = mean_i clip(d_i, 1e-12, 1e12)
(clip is a no-op for this input distribution; d_i ~ 256 >> 1e-12).

Final design ("sorted one-hot matmul", engine-balanced):
  Rows host-sorted into 8 buckets by label rank r = l >> 7 (128 classes),
  padded to a fixed 1152 rows/bucket; the center gather becomes a dense fp8
  matmul against a host-built one-hot (K=128 classes):
     gT = C_b^T @ OHT                               (PE, fp8)
  Per-bucket combine+square, split to balance engines:
   * buckets 0..2 (PE path): a second accumulating matmul against -I puts
     d = (c - x) straight into PSUM; ACT squares from PSUM with accum_out.
   * buckets 3..7 (DVE path): DVE subtract (x fp8 - PSUM f32 -> bf16), then
     ACT square+accum from SBUF.
  (DVE cannot square PSUM: the ISA allows only one PSUM read per
  instruction, and tensor_tensor_reduce / tensor_scalar(pow) both failed;
  ACT ends up doing all 8 squares at ~1.2us/bucket, which together with the
  ~8us framework startup and ~4us teardown sets the current floor.)
  Final cross-partition reduce on host via the [128, 8] accumulator.

Per-core layouts (ROWS=8192 -> RPAD=9216 = 8*1152, D=128):
  xt  [128, 9216] fp8 : xt[f, i] = x_sorted[i, f]  (0 for pad rows)
  oht [128, 9216] fp8 : oht[c, i] = 1 iff label_sorted[i] == (i//1152)*128+c
  csb [128, 1024] fp8 : csb[c, r*128 + f] = centers[r*128 + c, f]
  nid [128,  128] fp8 : -I
fp8(e4m3) quantization of x and centers costs ~8e-4 rel error, well under
the 2e-2 gate.
"""

import numpy as np
import ml_dtypes

import concourse.bacc as bacc
import concourse.bass as bass
import concourse.tile as tile
from concourse import mybir
from concourse.bass_utils import run_bass_kernel_spmd

N, C, D = 65536, 1000, 128
N_CORES = 8
P = 128
ROWS_PER_CORE = N // N_CORES            # 8192
NB = 8                                  # buckets (label >> 7)
BROWS = 1152                            # rows per bucket after padding
RPAD = NB * BROWS                       # 9216
CPAD = 1024
CH_OFF = (0, 512, 1024)                 # matmul slice offsets within a bucket
CH_N = (512, 512, 128)                  # slice sizes (PSUM bank = 512 f32)

PE_PATH = (0, 1, 2)                      # buckets: PE -I matmul + ACT square from PSUM
                                         # buckets 3..7: DVE subtract + ACT square (bf16)

FP8 = ml_dtypes.float8_e4m3

_NC = None


def _build_nc():
    f32 = mybir.dt.float32
    bf16 = mybir.dt.bfloat16
    fp8 = mybir.dt.float8e4
    nc = bacc.Bacc(trn_type="TRN2")

    xt = nc.dram_tensor("xt", [P, RPAD], fp8, kind="ExternalInput")
    oht = nc.dram_tensor("oht", [P, RPAD], fp8, kind="ExternalInput")
    csb = nc.dram_tensor("csb", [P, CPAD], fp8, kind="ExternalInput")
    nid = nc.dram_tensor("nid", [P, P], fp8, kind="ExternalInput")
    out = nc.dram_tensor("out", [P, 6], f32, kind="ExternalOutput")

    with tile.TileContext(nc) as tc:
        with (
            tc.tile_pool(name="big", bufs=1) as big,
            tc.tile_pool(name="small", bufs=1) as small,
            tc.tile_pool(name="psp", bufs=2, space="PSUM") as psp,
        ):
            csb_sb = small.tile([P, CPAD], fp8)
            nid_sb = small.tile([P, P], fp8)
            xt_sb = big.tile([P, RPAD], fp8, tag="xt")
            oht_sb = big.tile([P, RPAD], fp8, tag="oht")
            d_sb = big.tile([P, 5 * BROWS], bf16, tag="d")

            # quarter-granular, interleaved across the two HWDGE engines;
            # csb and oht quarter 0 stream concurrently on different queues
            QCOLS = 2 * BROWS
            nc.sync.dma_start(out=csb_sb[:], in_=csb.ap())
            nc.scalar.dma_start(out=oht_sb[:, :QCOLS], in_=oht.ap()[:, :QCOLS])
            nc.sync.dma_start(out=xt_sb[:, :QCOLS], in_=xt.ap()[:, :QCOLS])
            nc.scalar.dma_start(out=nid_sb[:], in_=nid.ap())
            for q in range(1, 4):
                qs = slice(q * QCOLS, (q + 1) * QCOLS)
                e_oht = nc.sync if q % 2 == 1 else nc.scalar
                e_xt = nc.scalar if q % 2 == 1 else nc.sync
                e_oht.dma_start(out=oht_sb[:, qs], in_=oht.ap()[:, qs])
                e_xt.dma_start(out=xt_sb[:, qs], in_=xt.ap()[:, qs])

            acc = small.tile([P, 6], f32)
            for b in range(NB):
                ps = psp.tile([P, BROWS], f32)
                pe_sub = b in PE_PATH
                for k in range(3):
                    o = b * BROWS + CH_OFF[k]
                    n = CH_N[k]
                    ks = slice(CH_OFF[k], CH_OFF[k] + n)
                    nc.tensor.matmul(
                        out=ps[:, ks],
                        lhsT=csb_sb[:, b * P:(b + 1) * P],
                        rhs=oht_sb[:, o:o + n],
                        start=True, stop=not pe_sub,
                    )
                    if pe_sub:
                        nc.tensor.matmul(
                            out=ps[:, ks],
                            lhsT=nid_sb[:],
                            rhs=xt_sb[:, o:o + n],
                            start=False, stop=True,
                        )
                if pe_sub:
                    # PSUM holds (c - x); ACT squares straight out of PSUM
                    nc.scalar.activation(
                        out=ps[:],
                        in_=ps[:],
                        func=mybir.ActivationFunctionType.Square,
                        accum_out=acc[:, b:b + 1],
                    )
                else:
                    # DVE subtract to SBUF bf16; ACT squares fused over
                    # 2-bucket spans (d slices are contiguous) to amortize
                    # the ~0.8us fixed cost per ACTIVATE+accumulator-read
                    bs = slice(b * BROWS, (b + 1) * BROWS)
                    dj = d_sb[:, (b - 3) * BROWS:(b - 2) * BROWS]
                    nc.vector.tensor_tensor(
                        out=dj, in0=xt_sb[:, bs], in1=ps[:],
                        op=mybir.AluOpType.subtract,
                    )
                    if b in (4, 6, 7):
                        a_i = {4: 3, 6: 4, 7: 5}[b]
                        lo = {4: 0, 6: 2, 7: 4}[b] * BROWS
                        hi = (b - 2) * BROWS
                        df = d_sb[:, lo:hi]
                        nc.scalar.activation(
                            out=df, in_=df,
                            func=mybir.ActivationFunctionType.Square,
                            accum_out=acc[:, a_i:a_i + 1],
                        )
                if b == 4:
                    # first half of the result ships while buckets 5-7 run
                    nc.sync.dma_start(out=out.ap()[:, :4], in_=acc[:, :4])

            nc.scalar.dma_start(out=out.ap()[:, 4:], in_=acc[:, 4:])

    nc.compile()
    return nc


def _get_nc():
    global _NC
    if _NC is None:
        _NC = _build_nc()
    return _NC


def make_in_maps(x, labels, centers):
    x = np.asarray(x, dtype=np.float32)
    labels_np = np.asarray(labels).astype(np.int64)
    centers = np.asarray(centers, dtype=np.float32)

    c_pad = np.zeros((CPAD, D), dtype=np.float32)
    c_pad[:C] = centers
    csb = np.ascontiguousarray(
        c_pad.reshape(NB, P, D).transpose(1, 0, 2).reshape(P, NB * D)
    ).astype(FP8)
    nid = (-np.eye(P, dtype=np.float32)).astype(FP8)

    in_maps = []
    for m in range(N_CORES):
        lo = m * ROWS_PER_CORE
        xc = x[lo:lo + ROWS_PER_CORE]
        lab = labels_np[lo:lo + ROWS_PER_CORE]
        rank = lab >> 7
        order = np.argsort(rank, kind="stable")
        counts = np.bincount(rank, minlength=NB)
        assert counts.max() <= BROWS, f"bucket overflow: {counts.max()} > {BROWS}"
        cum = np.concatenate([[0], np.cumsum(counts)])

        xs = np.zeros((RPAD, D), dtype=np.float32)
        cls_arr = np.full(RPAD, -1, dtype=np.int64)
        for b in range(NB):
            rows_b = order[cum[b]:cum[b + 1]]
            dst = b * BROWS
            xs[dst:dst + len(rows_b)] = xc[rows_b]
            cls_arr[dst:dst + len(rows_b)] = lab[rows_b] & 127

        oht = np.zeros((P, RPAD), dtype=FP8)
        valid = np.nonzero(cls_arr >= 0)[0]
        oht[cls_arr[valid], valid] = FP8(1.0)

        in_maps.append({
            "xt": np.ascontiguousarray(xs.T.astype(FP8)),
            "oht": np.ascontiguousarray(oht),
            "csb": csb,
            "nid": nid,
        })
    return in_maps


def run(x, labels, centers, **spmd_kwargs):
    """Run on the 8 NeuronCores; returns (loss, BassKernelResults)."""
    nc = _get_nc()
    in_maps = make_in_maps(x, labels, centers)
    res = run_bass_kernel_spmd(nc, in_maps, core_ids=list(range(N_CORES)), **spmd_kwargs)
    total = sum(float(np.asarray(r["out"], dtype=np.float64).sum()) for r in res.results)
    return np.float32(total / N), res


def kernel(x, labels, centers):
    loss, _ = run(x, labels, centers)
    return loss
